# revision 61
# baseline (speedup 1.0000x reference)
"""Trainium2 Bass kernel for nn_BrainGeneratorModel (bias-field corrupt + per-sample
separable Gaussian blur + label LUT remap), 8-core data/spatial parallel.

Sharding: 8 cores = (sample b in 0..3) x (H-half in 0..1). Each core processes a
[D=192, H=96 (+12 one-sided halo -> 108-row slab), W=192] subvolume plus labels.

Per-core image pipeline (blurs ride the transposes as regular matmuls):
  A) per d-batch (8 planes): bias matmul (K=4) -> exp (ACT) -> x*expb (DVE)
     -> T1' fused H-blur: matmul(lhsT=xb window [108h,108w], rhs=Gh [108h,96h'])
        -> psum [108w, 96h'] -> y2 windows resident in SBUF bf16 (w NOT blurred)
  B) per h-batch (8 rows): T2' fused W-blur: matmul(lhsT=y2win [108w, 108d],
     rhs=Gw [108w, 96w'']) -> zd [108d, (hl,w'')] -> banded D-blur matmul
     -> img out bf16.
  L) labels split three ways:
     - DVE 16-entry packed-int16 LUT (compare chain + OR accumulate)
     - PE: 32 one-hot compares (bf16-exact values) + identity-matmul PSUM accum
     - GPSIMD ap_gather with host-packed PAIR codes (l0+32*l1) into a
       1024-entry int32 table (T[l0] | T[l1]<<16) -> 2x gather throughput.
"""

import sys

for _p in ("/opt/trn_rl_repo",):
    if _p not in sys.path:
        sys.path.insert(0, _p)

import numpy as np
import ml_dtypes

import concourse.bass as bass
import concourse.mybir as mybir
import concourse.bacc as bacc
import concourse.tile as tile
from concourse.bass_utils import run_bass_kernel_spmd

F32 = mybir.dt.float32
BF16 = mybir.dt.bfloat16
I16 = mybir.dt.int16
I32 = mybir.dt.int32
A = mybir.AluOpType

B, C, D, H, W = 4, 1, 192, 192, 192
SMALL = 4
BIAS_STD = 0.7
MAX_SIGMA = 3.0
TRUNCATE = 4.0
K = 2 * int(TRUNCATE * MAX_SIGMA) + 1  # 25
P = K // 2  # 12
N_LABELS = 32
TABLE = 128

HC = 96            # interior H rows per core
HS = 108           # slab rows = HC + P (one-sided halo, edge-folded)
DB = 8             # d-batch size (stage A)
NB_A = D // DB     # 24 batches
HB = 8             # h-batch size (stage B)
NB_B = HC // HB    # 12 batches
FA = DB * W        # 1536 stage-A free size
WIN = 108          # banded blur contraction window (96 + 12)
FLAB = D * HC * W // 128  # 27648 label cols per partition

# --- label split across engines (cols) ---
FL_DVE = 4096      # DVE-direct share (16-entry packed LUT)
FL_PE = 8704       # PE share (32 scaled-one-hot matmul accumulation)
FL_G = FLAB - FL_DVE - FL_PE  # 14848 -> gpsimd pair-gather share
FL_G2 = FL_G // 2  # 7424 pair cols
LCH = 1024         # chunk cols for the DVE label path
PCH = 1024         # chunk cols for the PE label path
GCH = 128          # pair cols per ap_gather instruction (out free = 16*GCH i32)

_CACHE = {}

import os as _os
_NO_LABELS = bool(int(_os.environ.get("KERN_NO_LABELS", "0")))
_NO_IMG = bool(int(_os.environ.get("KERN_NO_IMG", "0")))


def _lin_weights(n_in, n_out):
    pos = np.linspace(0.0, n_in - 1.0, n_out, dtype=np.float64)
    i0 = np.clip(np.floor(pos).astype(np.int64), 0, n_in - 2)
    f = pos - i0
    Wm = np.zeros((n_out, n_in), np.float64)
    r = np.arange(n_out)
    np.add.at(Wm, (r, i0), 1.0 - f)
    np.add.at(Wm, (r, i0 + 1), f)
    return Wm


def _gauss_kernels(sigma3):
    ar = np.arange(K, dtype=np.float64) - K // 2
    out = np.zeros((3, K), np.float64)
    for i, sg in enumerate(sigma3):
        s = max(float(sg), 1e-3)
        g = np.exp(-0.5 * ar * ar / (s * s))
        g = g / g.sum()
        if float(sg) >= 0.01:
            out[i] = g
        else:
            out[i, K // 2] = 1.0
    return out


def _edge_folded_toeplitz(g, n):
    """[n, n] matrix M with out[j] = sum_i M[i, j] * x[i], replicate padding."""
    M = np.zeros((n, n), np.float64)
    for j in range(n):
        for t in range(K):
            src = min(max(j + t - P, 0), n - 1)
            M[src, j] += g[t]
    return M


def _build_program():
    nc = bacc.Bacc("TRN2", target_bir_lowering=False, debug=False)

    # ---- external inputs (per core) ----
    xs_h = nc.dram_tensor("xs", [HS, D * W], BF16, kind="ExternalInput")
    c_h = nc.dram_tensor("cydw", [4, D * W], BF16, kind="ExternalInput")
    wht_h = nc.dram_tensor("wht", [4, HS], BF16, kind="ExternalInput")
    gh_h = nc.dram_tensor("gh", [WIN, HC], BF16, kind="ExternalInput")
    gw0_h = nc.dram_tensor("gw0", [WIN, HC], BF16, kind="ExternalInput")
    gw1_h = nc.dram_tensor("gw1", [WIN, HC], BF16, kind="ExternalInput")
    gd0_h = nc.dram_tensor("gd0", [WIN, HC], BF16, kind="ExternalInput")
    gd1_h = nc.dram_tensor("gd1", [WIN, HC], BF16, kind="ExternalInput")
    lab_h = nc.dram_tensor("lab", [128, FL_DVE + FL_PE], I16, kind="ExternalInput")
    pcg_h = nc.dram_tensor("pcg", [128, FL_G2], I16, kind="ExternalInput")
    c16_h = nc.dram_tensor("c16", [128, 16], F32, kind="ExternalInput")
    tabf_h = nc.dram_tensor("tabf", [128, N_LABELS], F32, kind="ExternalInput")
    tab2_h = nc.dram_tensor("tab2", [128, 1024], I32, kind="ExternalInput")
    idbf_h = nc.dram_tensor("idbf", [128, 128], BF16, kind="ExternalInput")

    # ---- external outputs ----
    img_h = nc.dram_tensor("img", [HC, NB_B * 2 * FA], BF16, kind="ExternalOutput")
    labo_h = nc.dram_tensor("labo", [128, FL_DVE], I16, kind="ExternalOutput")
    labp_h = nc.dram_tensor("labp", [128, FL_PE], BF16, kind="ExternalOutput")
    labg_h = nc.dram_tensor("labg", [8, 16 * FL_G2], I32, kind="ExternalOutput")

    from contextlib import ExitStack
    with tile.TileContext(nc) as tc:
        with ExitStack() as stack:
            cst = stack.enter_context(tc.tile_pool(name="consts", bufs=1))
            sxp = stack.enter_context(tc.tile_pool(name="sxp", bufs=2))
            cbp = stack.enter_context(tc.tile_pool(name="cbp", bufs=2))
            ebp = stack.enter_context(tc.tile_pool(name="ebp", bufs=2))
            y2p = stack.enter_context(tc.tile_pool(name="y2p", bufs=1))
            zdp = stack.enter_context(tc.tile_pool(name="zdp", bufs=2))
            zip_ = stack.enter_context(tc.tile_pool(name="zip", bufs=3))
            lp = stack.enter_context(tc.tile_pool(name="lp", bufs=2))
            ltmp = stack.enter_context(tc.tile_pool(name="ltmp", bufs=1))
            dkp = stack.enter_context(tc.tile_pool(name="dkp", bufs=9))
            obp = stack.enter_context(tc.tile_pool(name="obp", bufs=1))
            glp = stack.enter_context(tc.tile_pool(name="glp", bufs=1))
            gop = stack.enter_context(tc.tile_pool(name="gop", bufs=4))
            psAp = stack.enter_context(tc.tile_pool(name="psA", bufs=2, space="PSUM"))
            psBp = stack.enter_context(tc.tile_pool(name="psB", bufs=2, space="PSUM"))
            psLp = stack.enter_context(tc.tile_pool(name="psL", bufs=2, space="PSUM"))

            # ---- constants to SBUF ----
            whtt = cst.tile([4, HS], BF16)
            nc.sync.dma_start(whtt[:], wht_h.ap())
            ght = cst.tile([WIN, HC], BF16)
            nc.sync.dma_start(ght[:], gh_h.ap())
            tab2 = cst.tile([128, 1024], I32)
            ltg = glp.tile([128, FL_G2], I16, tag="ltg")
            c16t = cst.tile([128, 16], F32)
            tabf = cst.tile([128, N_LABELS], F32)
            idbf = cst.tile([128, 128], BF16)
            gw0 = cst.tile([WIN, HC], BF16)
            gw1 = cst.tile([WIN, HC], BF16)
            gd0 = cst.tile([WIN, HC], BF16)
            gd1 = cst.tile([WIN, HC], BF16)

            def emit_big_consts():
                """Deferred: issued after the first batch's input loads. The
                first small ltg chunk lets gathers start ~4us earlier."""
                nc.sync.dma_start(tab2[:], tab2_h.ap())
                nc.sync.dma_start(ltg[:, :1024], pcg_h.ap()[:, :1024])
                nc.sync.dma_start(c16t[:], c16_h.ap())
                nc.sync.dma_start(tabf[:], tabf_h.ap())
                nc.sync.dma_start(idbf[:], idbf_h.ap())
                nc.sync.dma_start(ltg[:, 1024:FL_G2 // 2],
                                  pcg_h.ap()[:, 1024:FL_G2 // 2])
                nc.sync.dma_start(ltg[:, FL_G2 // 2:], pcg_h.ap()[:, FL_G2 // 2:])
                nc.sync.dma_start(gw0[:], gw0_h.ap())
                nc.sync.dma_start(gw1[:], gw1_h.ap())
                nc.sync.dma_start(gd0[:], gd0_h.ap())
                nc.sync.dma_start(gd1[:], gd1_h.ap())

            # y2 SBUF-resident: two w-windows [108, (d, h')] bf16, UNBLURRED in w
            y2w0 = y2p.tile([WIN, D * HC], BF16, tag="y2w0")
            y2w1 = y2p.tile([WIN, D * HC], BF16, tag="y2w1")
            y2w = [y2w0, y2w1]

            # ============ label path generators (fine-grained drip) ============
            def gen_label_dve():
                """DVE 16-entry packed LUT; yields after every instruction."""
                for s0 in range(0, FL_DVE, LCH):
                    fc = min(LCH, FL_DVE - s0)
                    lt = lp.tile([128, LCH], I16, tag="lt")
                    nc.sync.dma_start(lt[:, :fc], lab_h.ap()[:, s0:s0 + fc])
                    yield
                    hh = ltmp.tile([128, LCH], I16, tag="hh")
                    sh = ltmp.tile([128, LCH], I16, tag="sh")
                    acc = ltmp.tile([128, LCH], I16, tag="acc")
                    ek = ltmp.tile([128, LCH], I16, tag="ek")
                    nc.vector.tensor_scalar(hh[:, :fc], lt[:, :fc], 1, None,
                                            A.logical_shift_right)
                    yield
                    nc.vector.tensor_scalar(sh[:, :fc], lt[:, :fc], 1, 3,
                                            A.bitwise_and, A.logical_shift_left)
                    yield
                    nc.vector.tensor_scalar(acc[:, :fc], hh[:, :fc], 0,
                                            c16t[:, 0:1], A.is_equal, A.mult)
                    yield
                    for k in range(1, 16):
                        nc.vector.tensor_scalar(ek[:, :fc], hh[:, :fc], k,
                                                c16t[:, k:k + 1], A.is_equal, A.mult)
                        yield
                        nc.vector.tensor_tensor(acc[:, :fc], acc[:, :fc],
                                                ek[:, :fc], A.bitwise_or)
                        yield
                    o16 = lp.tile([128, LCH], I16, tag="o16")
                    nc.vector.tensor_tensor(o16[:, :fc], acc[:, :fc], sh[:, :fc],
                                            A.logical_shift_right)
                    yield
                    nc.sync.dma_start(labo_h.ap()[:, s0:s0 + fc], o16[:, :fc])
                    yield

            def gen_label_pe():
                """PE path: 32 one-hot compares (DVE) -> identity-matmul PSUM
                accumulation in 512-col rounds; ob copy + store on ACT."""
                LOOKAHEAD = 3
                for s0 in range(0, FL_PE, PCH):
                    fc = min(PCH, FL_PE - s0)
                    lt = lp.tile([128, PCH], I16, tag="ltp")
                    nc.sync.dma_start(lt[:, :fc], lab_h.ap()[:, FL_DVE + s0:
                                                               FL_DVE + s0 + fc])
                    yield
                    nq = (fc + 511) // 512
                    psls = []
                    for _ in range(nq):
                        pslt = psLp.tile([128, 512], F32, tag="psl")
                        psls.append(pslt)
                    dks = {}

                    def emit_compare(k):
                        dk = dkp.tile([128, PCH], BF16, tag="dk")
                        nc.vector.tensor_scalar(dk[:, :fc], lt[:, :fc], k,
                                                tabf[:, k:k + 1], A.is_equal, A.mult)
                        dks[k] = dk
                    for k in range(LOOKAHEAD):
                        emit_compare(k)
                    for k in range(N_LABELS):
                        if k + LOOKAHEAD < N_LABELS:
                            emit_compare(k + LOOKAHEAD)
                        dk = dks.pop(k)
                        for q in range(nq):
                            qn = min(512, fc - q * 512)
                            nc.tensor.matmul(psls[q][:, :qn], idbf[:],
                                             dk[:, q * 512:q * 512 + qn],
                                             start=(k == 0), stop=(k == N_LABELS - 1))
                        if k % 4 == 3:
                            yield
                    yield
                    ob = obp.tile([128, PCH], BF16, tag="ob")
                    for q in range(nq):
                        qn = min(512, fc - q * 512)
                        nc.scalar.copy(ob[:, q * 512:q * 512 + qn], psls[q][:, :qn])
                        yield
                    nc.scalar.dma_start(labp_h.ap()[:, s0:s0 + fc], ob[:, :fc])
                    yield

            def gen_label_gather():
                """GPSIMD pair-gather; og stores issued on SP 3 gathers behind
                so the in-order SP queue never blocks on Pool."""
                PEND = 3
                pend = []

                def flush_one():
                    og_p, g0_p, gc_p = pend.pop(0)
                    pstep = og_p[:].ap[0][0]
                    nc.sync.dma_start(
                        labg_h.ap()[:, 16 * g0_p:16 * (g0_p + gc_p)],
                        bass.AP(og_p.tensor, og_p[:].offset,
                                [[pstep * 16, 8], [1, 16 * gc_p]]),
                    )
                for g0 in range(0, FL_G2, GCH):
                    gc = min(GCH, FL_G2 - g0)
                    og = gop.tile([128, 16 * GCH], I32, tag="og")
                    nc.gpsimd.ap_gather(og[:, :16 * gc], tab2[:],
                                        ltg[:, g0:g0 + gc],
                                        channels=128, num_elems=1024, d=1,
                                        num_idxs=16 * gc)
                    if len(pend) >= PEND:
                        flush_one()
                    pend.append((og, g0, gc))
                    yield
                while pend:
                    flush_one()

            TOT = NB_A + 2 * NB_B
            gens = []
            if not _NO_LABELS:
                gdve = gen_label_dve()
                gpe = gen_label_pe()
                gg = gen_label_gather()
                # total steps per generator (for fraction-based dripping)
                n_dve = -(-FL_DVE // LCH) * 36
                n_pe = -(-FL_PE // PCH) * 13
                n_g = -(-FL_G2 // GCH)
                gens = [[gdve, n_dve, 0, 1.057], [gpe, n_pe, 0, 1.057],
                        [gg, n_g, 0, 1.057]]

            def drip(frac):
                for gen in gens:
                    f = min(1.0, frac * gen[3])
                    tgt = min(gen[1], int(round(f * gen[1])))
                    while gen[2] < tgt:
                        try:
                            next(gen[0])
                        except StopIteration:
                            gen[2] = gen[1]
                            break
                        gen[2] += 1

            # ================= stage A (software-pipelined) =================
            def phase_a0(ib):
                """input loads only (runs 2 batches ahead)."""
                d0 = ib * DB
                sx = sxp.tile([HS, FA], BF16)
                nc.sync.dma_start(sx[:], xs_h.ap()[:, d0 * W:(d0 + DB) * W])
                cb = cbp.tile([4, FA], BF16)
                nc.sync.dma_start(cb[:], c_h.ap()[:, d0 * W:(d0 + DB) * W])
                return sx, cb

            def phase_a1(sx, cb):
                """bias matmul + exp."""
                eb = ebp.tile([HS, FA], BF16, tag="eb")
                for q in range(FA // 512):
                    sl = slice(q * 512, (q + 1) * 512)
                    psb = psBp.tile([HS, 512], F32, tag="psB")
                    nc.tensor.matmul(psb[:], whtt[:], cb[:, sl], start=True, stop=True)
                    nc.scalar.activation(eb[:, sl], psb[:],
                                         mybir.ActivationFunctionType.Exp)
                return sx, eb

            def phase_a2(ib, sx, eb):
                """mult (in-place into eb), fused H-blur transposes -> y2."""
                d0 = ib * DB
                xb = eb
                for q in range(FA // 512):
                    sl = slice(q * 512, (q + 1) * 512)
                    nc.vector.tensor_tensor(xb[:, sl], sx[:, sl], eb[:, sl], A.mult)

                for win in range(2):
                    pt = psAp.tile([WIN, 1024], F32, tag="psA")
                    for t in range(DB):
                        nc.tensor.matmul(
                            pt[:, t * 128:t * 128 + HC],
                            xb[:, t * W + 84 * win: t * W + 84 * win + WIN],
                            ght[:], start=True, stop=True)
                    pstep = pt[:].ap[0][0]
                    nc.scalar.copy(
                        y2w[win][:, ib * DB * HC:(ib + 1) * DB * HC],
                        bass.AP(pt.tensor, pt[:].offset,
                                [[pstep, WIN], [128, DB], [1, HC]]),
                    )

            # ================= stage B (software-pipelined) =================
            def phase_b1(jb):
                """fused W-blur transposes -> zd windows [108d, (hl, w'')]."""
                h0 = jb * HB
                zda = zdp.tile([WIN, HB * W], BF16, tag="zd0")
                zdb = zdp.tile([WIN, HB * W], BF16, tag="zd1")
                zd = [zda, zdb]
                for r in range(2):
                    pt2a = psAp.tile([WIN, 1024], F32, tag="psA")
                    pt2b = psAp.tile([WIN, 1024], F32, tag="psA")
                    pts = [pt2a, pt2b]
                    for t in range(4):
                        hl = r * 4 + t
                        for m in range(2):
                            for dwin in range(2):
                                lhs = bass.AP(
                                    y2w[m].tensor,
                                    y2w[m][:].offset + 84 * dwin * HC + h0 + hl,
                                    [[y2w[m][:].ap[0][0], WIN], [HC, WIN]])
                                nc.tensor.matmul(
                                    pts[dwin][:, (t * 2 + m) * 128:
                                              (t * 2 + m) * 128 + HC],
                                    lhs, (gw0, gw1)[m][:],
                                    start=True, stop=True)
                    for dwin in range(2):
                        pstep = pts[dwin][:].ap[0][0]
                        cpy = nc.vector.tensor_copy if jb >= 12 else nc.scalar.copy
                        cpy(
                            bass.AP(zd[dwin].tensor,
                                    zd[dwin][:].offset + r * 4 * W,
                                    [[zd[dwin][:].ap[0][0], WIN],
                                     [W, 4], [HC, 2], [1, HC]]),
                            bass.AP(pts[dwin].tensor, pts[dwin][:].offset,
                                    [[pstep, WIN], [256, 4], [128, 2], [1, HC]]),
                        )
                return zd

            def phase_b2(jb, zd):
                """banded D-blur + img out; d-half m covers d' in [96m, 96m+96)."""
                for m, gdm in enumerate((gd0, gd1)):
                    zi = zip_.tile([HC, FA], BF16, tag="zi")
                    for q in range(3):
                        sl = slice(q * 512, (q + 1) * 512)
                        psd = psBp.tile([HS, 512], F32, tag="psB")
                        nc.tensor.matmul(psd[0:HC, :], gdm[:], zd[m][:, sl],
                                         start=True, stop=True)
                        if jb >= 9:
                            nc.vector.tensor_copy(zi[:, sl], psd[0:HC, :])
                        else:
                            nc.scalar.copy(zi[:, sl], psd[0:HC, :])
                    nc.scalar.dma_start(
                        img_h.ap()[:, (jb * 2 + m) * FA:(jb * 2 + m + 1) * FA],
                        zi[:])

            # ---- emission with drip interleaving (B slots weighted 2x) ----
            if not _NO_IMG:
                # a0 loads run 2 batches ahead; a1 compute 1 ahead; a2 consumes
                q0 = [phase_a0(0)]
                emit_big_consts()
                q0.append(phase_a0(1))
                drip(2.0 / TOT)
                p1 = phase_a1(*q0.pop(0))
                for ib in range(NB_A):
                    if ib + 2 < NB_A:
                        q0.append(phase_a0(ib + 2))
                    p1_next = phase_a1(*q0.pop(0)) if ib + 1 < NB_A else None
                    phase_a2(ib, *p1)
                    drip((ib + 1) / TOT)
                    p1 = p1_next
            else:
                emit_big_consts()
                drip(1.0 / TOT)

            pendb = None
            for jb in range(NB_B + 1 if not _NO_IMG else 0):
                curb = phase_b1(jb) if jb < NB_B else None
                if pendb is not None:
                    phase_b2(jb - 1, pendb)
                    drip((NB_A + 2 * (jb + 1)) / TOT)
                pendb = curb

            # exhaust all label generators (emission epilogues included)
            for gen in gens:
                while True:
                    try:
                        next(gen[0])
                    except StopIteration:
                        break

    nc.compile()
    return nc


def _host_prep(x, small_bias, sigma01, labels, source_values, dest_values):
    Wd = _lin_weights(SMALL, D)
    Whm = _lin_weights(SMALL, H)
    Wwm = _lin_weights(SMALL, W)
    eyebf = np.eye(128, dtype=ml_dtypes.bfloat16)

    mapping = np.zeros(TABLE, np.int64)
    mapping[np.asarray(source_values, np.int64)] = np.asarray(dest_values, np.int64)
    T = mapping[:N_LABELS]
    C16 = (T[0::2] | (T[1::2] << 8)).astype(np.float32)
    c16_rep = np.broadcast_to(C16, (128, 16)).copy()
    tabf_rep = np.broadcast_to(T.astype(np.float32), (128, N_LABELS)).copy()
    pc = np.arange(1024)
    tab2 = (T[pc & 31] | (T[pc >> 5] << 16)).astype(np.int32)
    tab2_rep = np.broadcast_to(tab2, (128, 1024)).copy()

    in_maps = []
    for c in range(8):
        b, half = c // 2, c % 2
        h0 = half * HC
        off = half * (H - HS)  # slab rows h in [off, off+108)
        hidx = np.arange(off, off + HS)

        # x slab, h-major layout [HS, D, W] in bf16
        xs = np.ascontiguousarray(
            np.asarray(x[b, 0], np.float32)[:, hidx, :].transpose(1, 0, 2)
        ).astype(ml_dtypes.bfloat16).reshape(HS, D * W)

        sm = np.asarray(small_bias[b, 0], np.float64) * BIAS_STD
        Cydw = np.einsum("xyz,dx,wz->ydw", sm, Wd, Wwm).reshape(4, D * W)
        WhT = np.ascontiguousarray(Whm[hidx, :].T)

        g3 = _gauss_kernels(np.asarray(sigma01[b], np.float64) * MAX_SIGMA)
        Mh = _edge_folded_toeplitz(g3[1], H)
        Mw = _edge_folded_toeplitz(g3[2], W)
        Md = _edge_folded_toeplitz(g3[0], D)
        Gh = Mh[off:off + WIN, h0:h0 + HC]
        Gw0 = Mw[0:WIN, 0:HC]
        Gw1 = Mw[84:192, HC:192]
        Gd0 = Md[0:WIN, 0:HC]
        Gd1 = Md[84:192, HC:192]

        lab = np.asarray(labels[b, 0][:, h0:h0 + HC, :], np.int16).reshape(128, FLAB)
        labdp = np.ascontiguousarray(lab[:, :FL_DVE + FL_PE])
        lg = lab[:, FL_DVE + FL_PE:].astype(np.int16)
        pcg = (lg[:, 0::2] + 32 * lg[:, 1::2]).astype(np.int16)

        in_maps.append({
            "xs": xs,
            "cydw": Cydw.astype(ml_dtypes.bfloat16),
            "wht": WhT.astype(ml_dtypes.bfloat16),
            "gh": Gh.astype(ml_dtypes.bfloat16),
            "gw0": Gw0.astype(ml_dtypes.bfloat16),
            "gw1": Gw1.astype(ml_dtypes.bfloat16),
            "gd0": Gd0.astype(ml_dtypes.bfloat16),
            "gd1": Gd1.astype(ml_dtypes.bfloat16),
            "lab": labdp,
            "pcg": np.ascontiguousarray(pcg),
            "c16": c16_rep,
            "tabf": tabf_rep,
            "tab2": tab2_rep,
            "idbf": eyebf,
        })
    return in_maps


def kernel(x, small_bias, sigma01, labels, source_values, dest_values):
    if "nc" not in _CACHE:
        _CACHE["nc"] = _build_program()
    nc = _CACHE["nc"]

    in_maps = _host_prep(x, small_bias, sigma01, labels, source_values, dest_values)
    res = run_bass_kernel_spmd(nc, in_maps, core_ids=list(range(8)))

    img = np.empty((B, C, D, H, W), np.float32)
    labels_out = np.empty((B, C, D, H, W), np.int32)
    for c in range(8):
        b, half = c // 2, c % 2
        h0 = half * HC
        r = res.results[c]
        # img rows d' in [0,96); cols = (jb, m, hl, w)
        rimg = np.asarray(r["img"], np.float32).reshape(HC, NB_B, 2, HB, W)
        # -> [m, d', jb, hl, w] -> [192, 96, 192]
        rimg = rimg.transpose(2, 0, 1, 3, 4).reshape(D, HC, W)
        img[b, 0, :, h0:h0 + HC, :] = rimg
        lo = np.empty((128, FLAB), np.int32)
        lo[:, :FL_DVE] = (r["labo"] & 255).astype(np.int32)
        lo[:, FL_DVE:FL_DVE + FL_PE] = np.asarray(r["labp"], np.float32).astype(np.int32)
        # gather share: labg [8, 16*FL_G2]; group g rows hold (s, p) wrap order
        lgr = r["labg"].astype(np.int64).reshape(8, FL_G2, 16)
        lop = np.empty((128, FL_G2), np.int64)
        for g in range(8):
            lop[16 * g:16 * g + 16, :] = lgr[g].T
        base = FL_DVE + FL_PE
        lo[:, base + 0::2] = (lop & 0xFFFF).astype(np.int32)
        lo[:, base + 1::2] = (lop >> 16).astype(np.int32)
        labels_out[b, 0, :, h0:h0 + HC, :] = lo.reshape(D, HC, W)
    return img, labels_out
